# revision 1
# baseline (speedup 1.0000x reference)
"""Trainium2 Bass kernel for nn_BaselineGNN (GNN message passing).

Strategy (8 NeuronCores, SPMD), v2:
  - Node-partition: core c owns dst nodes [c*12500, (c+1)*12500), 98 blocks
    of 128 (PAD_SLICE=12544).
  - m = relu(h0@Wm+bm) stored bf16; all PE-path tensors bf16 (fp32 matmuls
    run 2 passes on the PE; bf16 streams 1 col/cycle).
  - The m AllGather is split into 4 chunk-major collectives ([13,25,30,30]
    blocks) so chunks stream while phase 1 still computes later blocks.
    m_full chunk tensors are also the src-index buckets: each has
    <= 30720*8 = 30720 rows... (8*30*128=30720) < 32768 so dma_gather's
    int16 indices address any row of a chunk directly (no quarter split of
    the old kernel; 4 buckets/block fall out of the AG chunking).
  - Phase 2: per group of 7 dst blocks and per chunk-bucket q, ONE batched
    dma_gather call (queue=q; the 4 Q7 core-pairs generate descriptors for
    the 4 buckets concurrently).  Counts are static: interior padding uses
    idx 0 (a valid row) whose dst one-hot column is -1 -> contributes 0.
    No reg_loads, no per-bucket calls; gathers prefetch 2 groups ahead so
    the PE streams back-to-back (HAM stays warm).
  - Scatter-add per block: aggrT accumulates in PSUM as
      m_blk^T (self loops, identity matmul)
      + sum_chunks mg[128e,H]^T @ onehot(dst)[128e,128d]
      + Ws^T @ h0T
    then hT = relu(aggrT+bs) (bf16), outT = Wo^T@hT+bo into an SBUF
    accumulator, DMA'd once at the end.
"""
import contextlib
import ctypes
import os
import sys

sys.path.insert(0, "/opt/trn_rl_repo")

import numpy as np
import ml_dtypes

import concourse.bass as bass
import concourse.bacc as bacc
import concourse.tile as tile
from concourse import mybir
from concourse.library_config import mlp
from concourse.masks import make_identity

N_NODES, N_EDGES, N_GRAPHS = 100000, 1600000, 1024
IN_LOCAL, IN_GLOBAL, HIDDEN, NUM_CLASSES = 16, 8, 128, 2
P = 128
N_CORES = 8
SLICE = N_NODES // N_CORES            # 12500
NBLK = 104                            # blocks per core (balanced, ~120/blk)
PAD_SLICE = NBLK * P                  # 13312
GBLK = N_GRAPHS // P                  # 8
UGRP = 8                              # phase-1 groups
UBLK = NBLK // UGRP                   # 13 blocks per group
UIDX = UBLK * P                       # 1664 u-gather idxs per call

CHUNKS = (26, 26, 26, 26)             # AG chunk sizes in dst blocks
CH_START = (0, 26, 52, 78)
NCHUNK = 4
RB = CHUNKS[0]                        # blocks per chunk region
G_BLKS = 8                            # phase-2 gather group size (blocks)
NGRP = NBLK // G_BLKS                 # 13

f32 = mybir.dt.float32
bf16 = mybir.dt.bfloat16
i16 = mybir.dt.int16
np_bf16 = ml_dtypes.bfloat16

_prog_cache: dict = {}
last_run: dict = {}


# --------------------------------------------------------------------------
# device program
# --------------------------------------------------------------------------
def _build(k_used):
    """k_used: [NBLK][NCHUNK] chunk counts (static schedule, all cores)."""
    k_used = np.asarray(k_used)
    tch = k_used.sum(axis=1)                      # one-hot cols per block
    TCHMAX = int(tch.max())
    boff = np.concatenate([[0], np.cumsum(tch)])  # dstT col offset per block
    TOTCH = int(boff[-1])
    # per (group, chunk-bucket) gather call capacities (in 128-chunks)
    gcap = np.zeros((NGRP, NCHUNK), np.int64)
    for g in range(NGRP):
        gcap[g] = k_used[g * G_BLKS:(g + 1) * G_BLKS].sum(axis=0)
    CAPM = [int(gcap[:, q].max()) for q in range(NCHUNK)]
    KM = [int(k_used[:, q].max()) for q in range(NCHUNK)]
    # idx-table col offset (in int16/16 cols) per call, group-major
    ixoff = np.zeros((NGRP, NCHUNK + 1), np.int64)
    off = 0
    for g in range(NGRP):
        for q in range(NCHUNK):
            ixoff[g][q] = off
            off += gcap[g][q] * 8        # cap*128 idxs -> /16 cols
        ixoff[g][NCHUNK] = off
    IXCOLS = int(off)
    IXGW = int(max(ixoff[g][NCHUNK] - ixoff[g][0] for g in range(NGRP)))

    nc = bacc.Bacc("TRN2", target_bir_lowering=False, debug=False,
                   num_devices=N_CORES, num_swdge_queues=4,
                   dynamic_dma_scratch_size=32768)

    def inp(name, shape, dt):
        return nc.dram_tensor(name, shape, dt, kind="ExternalInput").ap()

    xT_d = inp("xT", [IN_LOCAL, PAD_SLICE], f32)
    gfT_d = inp("gfT", [IN_GLOBAL, N_GRAPHS], f32)
    ixu_d = inp("ixu", [P, UGRP * UIDX // 16], i16)
    ixe_d = inp("ixe", [P, IXCOLS], i16)
    dstT_d = inp("dstT", [P, TOTCH], bf16)
    iota_d = inp("iota", [P, TCHMAX * P], bf16)
    Wg_d = inp("Wg", [IN_GLOBAL, HIDDEN], f32)
    Wc1_d = inp("Wc1", [IN_LOCAL, HIDDEN], f32)
    Wc2_d = inp("Wc2", [HIDDEN, HIDDEN], f32)
    Wm_d = inp("Wm", [HIDDEN, HIDDEN], bf16)
    Ws_d = inp("Ws", [HIDDEN, HIDDEN], bf16)
    Wo_d = inp("Wo", [HIDDEN, NUM_CLASSES], bf16)
    bg_d = inp("bg_c", [HIDDEN, 1], f32)
    bc_d = inp("bc_b", [P, HIDDEN], f32)
    bm_d = inp("bm_b", [P, HIDDEN], f32)
    bs_d = inp("bs_c", [HIDDEN, 1], f32)
    bo_d = inp("bo_c", [NUM_CLASSES, 1], f32)
    idb_d = inp("idb", [P, P], bf16)
    outT_d = nc.dram_tensor("outT", [NUM_CLASSES, PAD_SLICE], f32,
                            kind="ExternalOutput").ap()

    u_buf = nc.dram_tensor("u_buf", [N_GRAPHS, HIDDEN], f32).ap()
    m_sl = [nc.dram_tensor(f"m_sl{k}", [CHUNKS[k] * P, HIDDEN], bf16).ap()
            for k in range(NCHUNK)]
    m_full = [nc.dram_tensor(f"m_full{k}",
                             [N_CORES * CHUNKS[k] * P, HIDDEN], bf16,
                             addr_space="Shared").ap()
              for k in range(NCHUNK)]

    AF = mybir.ActivationFunctionType
    OP = mybir.AluOpType
    chunk_of = np.zeros(NBLK, np.int64)
    for k in range(NCHUNK):
        chunk_of[CH_START[k]:CH_START[k] + CHUNKS[k]] = k

    with tile.TileContext(nc) as tc:
        with (
            tc.tile_pool(name="const", bufs=1) as cpool,
            tc.tile_pool(name="persist", bufs=1) as ppool,
            tc.tile_pool(name="work", bufs=3) as wpool,
            tc.tile_pool(name="sbig", bufs=3) as spool,
            tc.tile_pool(name="uexp", bufs=3) as upool,
            tc.tile_pool(name="xg", bufs=2) as xgpool,
            tc.tile_pool(name="mg", bufs=12) as mgpool,
            tc.tile_pool(name="ixg", bufs=3) as ixpool,
            tc.tile_pool(name="ps_a", bufs=3, space="PSUM") as ps_a,
            tc.tile_pool(name="ps_b", bufs=3, space="PSUM") as ps_b,
            tc.tile_pool(name="ps_t", bufs=1, space="PSUM") as ps_t,
            tc.tile_pool(name="ps_o", bufs=1, space="PSUM") as ps_o,
        ):
            nc.gpsimd.load_library(mlp)

            def ctile(name, ap, shape, dt):
                t = cpool.tile(shape, dt, tag=f"c_{name}")
                nc.sync.dma_start(t[:], ap[:])
                return t

            Wg_t = ctile("Wg", Wg_d, [IN_GLOBAL, HIDDEN], f32)
            Wc1f_t = ctile("Wc1", Wc1_d, [IN_LOCAL, HIDDEN], f32)
            Wc2_t = ctile("Wc2", Wc2_d, [HIDDEN, HIDDEN], f32)
            Wm_t = ctile("Wm", Wm_d, [HIDDEN, HIDDEN], bf16)
            Ws_t = ctile("Ws", Ws_d, [HIDDEN, HIDDEN], bf16)
            Wo_t = ctile("Wo", Wo_d, [HIDDEN, NUM_CLASSES], bf16)
            bg_t = ctile("bg", bg_d, [HIDDEN, 1], f32)
            bc_t = ctile("bc", bc_d, [P, HIDDEN], f32)
            bm_t = ctile("bm", bm_d, [P, HIDDEN], f32)
            bs_t = ctile("bs", bs_d, [HIDDEN, 1], f32)
            bo_t = ctile("bo", bo_d, [NUM_CLASSES, 1], f32)
            gfT_t = ctile("gfT", gfT_d, [IN_GLOBAL, N_GRAPHS], f32)
            idb_t = ctile("idb", idb_d, [P, P], bf16)
            iota_t = ctile("iota", iota_d, [P, TCHMAX * P], bf16)
            dstT_t = ctile("dstT", dstT_d, [P, TOTCH], bf16)

            ident = cpool.tile([P, P], f32)
            make_identity(nc, ident[:])

            ixu_t = ppool.tile([P, UGRP * UIDX // 16], i16)
            nc.sync.dma_start(ixu_t[:], ixu_d[:])

            h0T_t = ppool.tile([HIDDEN, PAD_SLICE], bf16)   # 3.2 MB
            m16_t = ppool.tile([P, PAD_SLICE], bf16)        # 3.2 MB

            # ---------------- phase 0: global encoder ----------------
            for g in range(GBLK):
                gsl = slice(g * P, (g + 1) * P)
                ps1 = ps_b.tile([P, P], f32, tag="pb")
                nc.tensor.matmul(out=ps1[:], lhsT=Wg_t[:], rhs=gfT_t[:, gsl],
                                 start=True, stop=True)
                rT = wpool.tile([P, P], f32, tag="rT")
                nc.scalar.activation(out=rT[:], in_=ps1[:], func=AF.Relu,
                                     bias=bg_t[:, :1])
                ps2 = ps_b.tile([P, P], f32, tag="pb")
                nc.tensor.matmul(out=ps2[:], lhsT=Wc2_t[:], rhs=rT[:],
                                 start=True, stop=True)
                uT = wpool.tile([P, P], f32, tag="uT")
                nc.vector.tensor_copy(out=uT[:], in_=ps2[:])
                ps3 = ps_t.tile([P, P], f32, tag="pt")
                nc.tensor.transpose(out=ps3[:], in_=uT[:], identity=ident[:])
                ub = wpool.tile([P, P], f32, tag="ublk")
                nc.vector.tensor_tensor(out=ub[:], in0=ps3[:], in1=bc_t[:],
                                        op=OP.add)
                nc.sync.dma_start(u_buf[gsl, :], ub[:])

            # ---------------- phase 1: h0 / m on own slice ----------------
            for uc in range(UGRP):
                xgf = xgpool.tile([IN_LOCAL, UBLK * P], f32, tag="xg")
                nc.sync.dma_start(xgf[:], xT_d[:, uc * UBLK * P:(uc + 1) * UBLK * P])
                uexp = upool.tile([P, UBLK, HIDDEN], f32, tag="uexp")
                for i2, (b0, b1) in enumerate(((0, 6), (6, UBLK))):
                    n2 = (b1 - b0) * P
                    nc.gpsimd.dma_gather(
                        uexp[:, b0:b1, :], u_buf[:],
                        ixu_t[:, uc * (UIDX // 16) + b0 * 8:
                              uc * (UIDX // 16) + b1 * 8],
                        n2, n2, HIDDEN,
                        single_packet=False, queue_num=(2 * uc + i2) % 4)
                for j in range(UBLK):
                    b = uc * UBLK + j
                    bsl = slice(b * P, (b + 1) * P)
                    jsl = slice(j * P, (j + 1) * P)
                    psh = ps_b.tile([P, P], f32, tag="pb")
                    nc.tensor.matmul(out=psh[:], lhsT=Wc1f_t[:],
                                     rhs=xgf[:, jsl], start=True, stop=False)
                    nc.tensor.matmul(out=psh[:], lhsT=uexp[:, j, :],
                                     rhs=ident[:], is_transpose=True,
                                     start=False, stop=True)
                    nc.vector.tensor_scalar_max(out=h0T_t[:, bsl],
                                                in0=psh[:], scalar1=0.0)
                    psm = ps_b.tile([P, P], f32, tag="pb")
                    nc.tensor.matmul(out=psm[:], lhsT=h0T_t[:, bsl],
                                     rhs=Wm_t[:], start=True, stop=True)
                    nc.vector.tensor_tensor(out=m16_t[:, bsl], in0=psm[:],
                                            in1=bm_t[:], op=OP.add)
                    nc.vector.tensor_scalar_max(out=m16_t[:, bsl],
                                                in0=m16_t[:, bsl], scalar1=0.0)
                    k = int(chunk_of[b])
                    rb = b - CH_START[k]
                    nc.sync.dma_start(m_sl[k][rb * P:(rb + 1) * P, :],
                                      m16_t[:, bsl])
                    # trigger the chunk's AllGather as soon as its last
                    # block is written (overlaps rest of phase 1)
                    if b == CH_START[k] + CHUNKS[k] - 1:
                        nc.gpsimd.collective_compute(
                            "AllGather", OP.bypass,
                            replica_groups=[list(range(N_CORES))],
                            ins=[m_sl[k][:]], outs=[m_full[k][:]])

            # ---------------- phase 2: scatter-add + update + readout ------
            iota_v = iota_t[:].rearrange("p (k f) -> p k f", k=TCHMAX)
            mg_tiles = {}
            self_call_ctr = [0]

            def emit_gathers(g):
                gw = int(ixoff[g][NCHUNK] - ixoff[g][0])
                ixg = ixpool.tile([P, IXGW], i16, tag="ixg")
                nc.sync.dma_start(
                    ixg[:, :gw],
                    ixe_d[:, int(ixoff[g][0]):int(ixoff[g][NCHUNK])])
                for q in range(NCHUNK):
                    # one call + tile per dst block; queue round-robins over
                    # calls (decoupled from the chunk bucket) so all 4
                    # queue-pairs stream as soon as the first AG chunk lands
                    c0 = int(ixoff[g][q] - ixoff[g][0])
                    seg = 0
                    for j in range(G_BLKS):
                        b = g * G_BLKS + j
                        k = int(k_used[b][q])
                        t = mgpool.tile([P, KM[q], HIDDEN], bf16,
                                        tag=f"mg{q}")
                        nc.gpsimd.dma_gather(
                            t[:, :k, :], m_full[q][:],
                            ixg[:, c0 + seg * 8:c0 + (seg + k) * 8],
                            k * P, k * P, HIDDEN,
                            single_packet=False,
                            queue_num=self_call_ctr[0] % 4)
                        self_call_ctr[0] += 1
                        seg += k
                        mg_tiles[(b, q)] = t

            emit_gathers(0)
            emit_gathers(1)
            for g in range(NGRP):
                if g + 2 < NGRP:
                    emit_gathers(g + 2)
                for j in range(G_BLKS):
                    b = g * G_BLKS + j
                    bsl = slice(b * P, (b + 1) * P)
                    tch = int(k_used[b].sum())
                    S = spool.tile([P, TCHMAX, P], bf16, tag="S")
                    nc.vector.tensor_tensor(
                        out=S[:, :tch, :],
                        in0=dstT_t[:, int(boff[b]):int(boff[b + 1])]
                            .to_broadcast([P, tch, P]),
                        in1=iota_v[:, :tch, :], op=OP.is_equal)
                    pa = ps_a.tile([HIDDEN, P], f32, tag="pa")
                    # self loops: aggrT += m_block^T
                    nc.tensor.matmul(out=pa[:], lhsT=m16_t[:, bsl],
                                     rhs=idb_t[:], start=True, stop=False)
                    col = 0
                    for q in range(NCHUNK):
                        mg = mg_tiles[(b, q)]
                        for c in range(int(k_used[b][q])):
                            nc.tensor.matmul(
                                out=pa[:], lhsT=mg[:, c, :],
                                rhs=S[:, col, :], start=False, stop=False)
                            col += 1
                    nc.tensor.matmul(out=pa[:], lhsT=Ws_t[:],
                                     rhs=h0T_t[:, bsl], start=False, stop=True)
                    hT = wpool.tile([HIDDEN, P], bf16, tag="hT")
                    nc.scalar.activation(out=hT[:], in_=pa[:], func=AF.Relu,
                                         bias=bs_t[:, :1])
                    po = ps_o.tile([NUM_CLASSES, P], f32, tag="po")
                    nc.tensor.matmul(out=po[:], lhsT=Wo_t[:], rhs=hT[:],
                                     start=True, stop=True)
                    ob = wpool.tile([NUM_CLASSES, P], f32, tag="ob")
                    nc.scalar.activation(out=ob[:], in_=po[:],
                                         func=AF.Identity, bias=bo_t[:, :1])
                    nc.sync.dma_start(outT_d[:, b * P:(b + 1) * P], ob[:])
                    for q in range(NCHUNK):
                        del mg_tiles[(b, q)]

    nc.compile()
    return nc


# --------------------------------------------------------------------------
# host side
# --------------------------------------------------------------------------
def _wrap16(ix):
    """dma_gather int16 index layout: [16, n/16] wrapped, tiled to 128 parts."""
    return np.tile(ix.reshape(-1, 16).T, (8, 1))


def _balance(dst_all, src_all):
    """Assign each core's nodes to NBLK blocks (<=128 each) so that every
    (block, src-chunk-region) in-degree bucket stays <= 4*128.  A node's
    chunk region (for its outgoing edges) is its original quarter of the
    core slice, so regions are fixed before balancing and the per-core
    problems decouple."""
    region_of = (np.arange(N_NODES) % SLICE) // (SLICE // NCHUNK)
    q_src = region_of[src_all]
    core_of = dst_all // SLICE
    reg_n = SLICE // NCHUNK
    pos_of = np.zeros((N_CORES, SLICE), np.int64)
    for c in range(N_CORES):
        sel = core_of == c
        d_loc = dst_all[sel] - c * SLICE
        deg = np.zeros((SLICE, NCHUNK), np.int32)
        np.add.at(deg, (d_loc, q_src[sel]), 1)
        for ch in range(NCHUNK):
            nodes = np.arange(ch * reg_n, (ch + 1) * reg_n)
            dv = deg[nodes]
            order = np.argsort(-dv.sum(axis=1), kind="stable")
            loads = np.zeros((RB, NCHUNK), np.int64)
            cnts = np.zeros(RB, np.int64)
            for n in order:
                score = (loads + dv[n]).max(axis=1) * 1024 + cnts
                score[cnts >= P] = 1 << 60
                bb = int(np.argmin(score))
                pos_of[c][nodes[n]] = (ch * RB + bb) * P + cnts[bb]
                loads[bb] += dv[n]
                cnts[bb] += 1
    return pos_of


def _preprocess(inputs):
    x = np.asarray(inputs["x"], dtype=np.float32)
    ei = np.asarray(inputs["edge_index"]).astype(np.int64)
    batch = np.asarray(inputs["batch"]).astype(np.int64)
    gf = np.asarray(inputs["global_feat"], dtype=np.float32)
    W = {k: np.ascontiguousarray(np.asarray(inputs[k], dtype=np.float32))
         for k in ("Wg", "bg", "Wc", "bc", "Wm", "bm", "Ws", "bs", "Wo", "bo")}

    src_all, dst_all = ei[0], ei[1]
    pos_of = _balance(dst_all, src_all)
    # src -> (chunk bucket, row within that chunk's m_full tensor)
    s_core = src_all // SLICE
    s_pos = pos_of[s_core, src_all % SLICE]
    s_blk = s_pos // P
    s_ch = s_blk // RB
    s_row = (s_core * (RB * P) + (s_blk - s_ch * RB) * P
             + s_pos % P).astype(np.int16)
    core_of = dst_all // SLICE

    per_core = []
    counts = np.zeros((N_CORES, NBLK, NCHUNK), np.int64)
    for c in range(N_CORES):
        sel = np.nonzero(core_of == c)[0]
        d_pos = pos_of[c][dst_all[sel] - c * SLICE]
        blk = d_pos // P
        q = s_ch[sel]
        key = blk * NCHUNK + q
        order = np.argsort(key, kind="stable")
        sel, key = sel[order], key[order]
        cnt = np.bincount(key, minlength=NBLK * NCHUNK).reshape(NBLK, NCHUNK)
        counts[c] = cnt
        per_core.append((key, (d_pos[order] % P).astype(np.float32),
                         s_row[sel], cnt))

    k_used = (-(-counts.max(axis=0) // P)).astype(np.int64)  # [NBLK, NCHUNK]
    tch = k_used.sum(axis=1)
    boff = np.concatenate([[0], np.cumsum(tch)])
    TOTCH = int(boff[-1])
    TCHMAX = int(tch.max())

    # static call layout: per (group, bucket) capacity and flat offsets
    gcap = np.zeros((NGRP, NCHUNK), np.int64)
    for g in range(NGRP):
        gcap[g] = k_used[g * G_BLKS:(g + 1) * G_BLKS].sum(axis=0)
    # flat idx-slot offset of bucket (b, q) inside the concatenated calls
    bucket_off = np.zeros((NBLK, NCHUNK), np.int64)
    off = 0
    for g in range(NGRP):
        for q in range(NCHUNK):
            for j in range(G_BLKS):
                b = g * G_BLKS + j
                bucket_off[b][q] = off
                off += k_used[b][q] * P
    TOTSLOT = int(off)

    iota_np = np.tile(np.arange(P, dtype=np.float32).astype(np_bf16),
                      (P, TCHMAX))
    shared = {
        "gfT": np.ascontiguousarray(gf.T),
        "iota": iota_np,
        "idb": np.eye(P, dtype=np.float32).astype(np_bf16),
        "Wg": W["Wg"],
        "Wc1": np.ascontiguousarray(W["Wc"][:IN_LOCAL]),
        "Wc2": np.ascontiguousarray(W["Wc"][IN_LOCAL:]),
        "Wm": W["Wm"].astype(np_bf16),
        "Ws": W["Ws"].astype(np_bf16),
        "Wo": W["Wo"].astype(np_bf16),
        "bg_c": W["bg"].reshape(HIDDEN, 1),
        "bc_b": np.tile(W["bc"], (P, 1)),
        "bm_b": np.tile(W["bm"], (P, 1)),
        "bs_c": W["bs"].reshape(HIDDEN, 1),
        "bo_c": W["bo"].reshape(NUM_CLASSES, 1),
    }

    in_maps = []
    for c in range(N_CORES):
        key, d128, l16, cnt = per_core[c]
        ix_pad = np.zeros(TOTSLOT, np.int16)
        dst_pad = np.full(TOTCH * P, -1.0, np.float32)
        flat_cnt = cnt.reshape(-1)
        cum = np.cumsum(flat_cnt) - flat_cnt
        within = np.arange(len(key)) - np.repeat(cum, flat_cnt)
        # gather-call slot (group-major) and dstT slot (block-major)
        pos_ix = np.repeat(bucket_off.reshape(-1), flat_cnt) + within
        dst_off = (boff[:-1][:, None]
                   + np.concatenate([np.zeros((NBLK, 1), np.int64),
                                     np.cumsum(k_used, axis=1)[:, :-1]],
                                    axis=1)) * P
        pos_dst = np.repeat(dst_off.reshape(-1), flat_cnt) + within
        ix_pad[pos_ix] = l16
        dst_pad[pos_dst] = d128
        ixe = np.concatenate(
            [_wrap16(ix_pad[bucket_off[g * G_BLKS][q]:
                            bucket_off[g * G_BLKS][q] + gcap[g][q] * P])
             for g in range(NGRP) for q in range(NCHUNK)], axis=1)
        dstT = np.ascontiguousarray(
            dst_pad.reshape(-1, P).T).astype(np_bf16)

        bpad = np.zeros(PAD_SLICE, np.int16)
        bpad[pos_of[c]] = batch[c * SLICE:(c + 1) * SLICE]
        ixu = np.concatenate(
            [_wrap16(bpad[g * UIDX:(g + 1) * UIDX]) for g in range(UGRP)],
            axis=1)

        xT = np.zeros((IN_LOCAL, PAD_SLICE), np.float32)
        xT[:, pos_of[c]] = x[c * SLICE:(c + 1) * SLICE].T

        m = dict(shared)
        m.update({"xT": xT, "ixu": ixu, "ixe": ixe, "dstT": dstT})
        in_maps.append(m)
    return k_used, pos_of, in_maps


# --------------------------------------------------------------------------
# profiling hook (NTFF via the axon PJRT .so; absent module in this image)
# --------------------------------------------------------------------------
def _profile_hook():
    so = "/opt/axon/libaxon_pjrt.so"
    if not os.path.exists(so):
        return None
    lib = ctypes.CDLL(so)
    if not hasattr(lib, "axon_start_nrt_profile"):
        return None
    lib.axon_start_nrt_profile.argtypes = [ctypes.POINTER(ctypes.c_int64),
                                           ctypes.c_size_t]
    lib.axon_start_nrt_profile.restype = ctypes.c_int64
    lib.axon_stop_nrt_profile.argtypes = [ctypes.c_char_p]
    lib.axon_stop_nrt_profile.restype = ctypes.c_int64

    @contextlib.contextmanager
    def hook(output_dir, device_ids):
        import jax
        jax.devices()
        if device_ids:
            ids = (ctypes.c_int64 * len(device_ids))(*device_ids)
            rc = lib.axon_start_nrt_profile(ids, len(device_ids))
        else:
            rc = lib.axon_start_nrt_profile(None, 0)
        if rc != 0:
            raise RuntimeError(f"axon_start_nrt_profile rc={rc}")
        try:
            yield
        finally:
            n = lib.axon_stop_nrt_profile(str(output_dir).encode())
            print(f"profile: {n} file(s) written to {output_dir}",
                  file=sys.stderr)

    return hook


def _run(nc, in_maps):
    from concourse import bass2jax
    trace_dir = os.environ.get("GNN_TRACE_DIR", "")
    if not trace_dir:
        return bass2jax.run_bass_via_pjrt(nc, in_maps, n_cores=N_CORES)
    hook = _profile_hook()
    if hook is None:
        return bass2jax.run_bass_via_pjrt(nc, in_maps, n_cores=N_CORES)
    import time as _time
    trace_dir = os.path.join(trace_dir, f"run_{int(_time.time()*1000)}")
    os.makedirs(trace_dir, exist_ok=True)
    last_run["trace_dir"] = trace_dir
    trace_cores = [int(t) for t in
                   os.environ.get("GNN_TRACE_CORES", "0").split(",")]
    with hook(trace_dir, trace_cores):
        results = bass2jax.run_bass_via_pjrt(nc, in_maps, n_cores=N_CORES)
    try:
        from concourse._compat import FishPath
        import gauge.profiler as gprof
        profile = gprof.Profile(
            profile_path=FishPath(trace_dir), kernel_dev_mode=True,
            profile_on_exit=False, bass_kernel=nc.m,
            offline_processing=True, fname="*_body*")
        profile.convert_ntffs_to_json(tuple(trace_cores))
        j = profile.load_json(trace_cores[0])
        last_run["summary"] = j["summary"][0] if j else None
        last_run["exec_time_ns"] = (
            int(j["summary"][0]["total_time"] * 1e9) if j else None)
        last_run["profile_json"] = str(profile.json_path(trace_cores[0]))
    except Exception as e:  # profiling must never break the run
        print(f"profile post-processing failed: {e}", file=sys.stderr)
    return results


def kernel(**inputs) -> np.ndarray:
    k_used, pos_of, in_maps = _preprocess(inputs)
    key = k_used.tobytes()
    nc = _prog_cache.get(key)
    if nc is None:
        nc = _build(k_used)
        _prog_cache[key] = nc
    last_run.clear()
    results = _run(nc, in_maps)
    out = np.empty((N_NODES, NUM_CLASSES), np.float32)
    for c in range(N_CORES):
        out[c * SLICE:(c + 1) * SLICE] = results[c]["outT"].T[pos_of[c]]
    return out



# revision 9
# speedup vs baseline: 1.1643x; 1.1643x over previous
"""Trainium2 Bass kernel for nn_BaselineGNN (GNN message passing).

Strategy (8 NeuronCores, SPMD), v3:
  - Node-partition: core c owns dst nodes [c*12500, (c+1)*12500), 104 blocks
    of 128 (PAD_SLICE=13312).
  - All PE-path tensors bf16 (fp32 matmuls run multiple passes on the PE).
  - Phase 1 computes h0/m with NO gpsimd work: u = relu(gf@Wg+bg)@Wc2+bc
    lives in SBUF as 8 [128-graph, H] tiles; per region (quarter core
    slice, < 128 distinct sorted graph ids) a per-core one-hot input SEL
    selects a 128-graph window uR, and per block the u[batch] term is ONE
    matmul uR @ OU with host-built one-hots.  ReLUs run on the Scalar
    engine; bm enters psm via a rank-1 ones x bm matmul.
  - m AllGather split into 4 chunk collectives ([26,26,26,26] blocks) that
    fire as soon as each chunk's m blocks are written (overlaps phase 1).
  - Phase 2: per (group of 8 dst blocks, chunk bucket q) ONE dma_gather of
    ~4096 rows on queue q.  queue q only ever waits on AllGather q, so the
    4 SWDGE queues stream independently with no head-of-line blocking.
    Padding slots use idx 0 (a valid row) with dst one-hot column -1 -> 0.
  - Scatter-add per block: aggrT accumulates in PSUM as
      m_blk^T (self loops, identity matmul)
      + sum_chunks mg[128e,H]^T @ onehot(dst)[128e,128d]
      + Ws^T @ h0T
    then hT = relu(aggrT+bs) (scalar engine), outT = Wo^T@hT+bo into an
    SBUF accumulator, DMA'd once at the end.
"""
import contextlib
import ctypes
import os
import sys

sys.path.insert(0, "/opt/trn_rl_repo")

import numpy as np
import ml_dtypes

import concourse.bass as bass
import concourse.bacc as bacc
import concourse.tile as tile
from concourse import mybir
from concourse.library_config import mlp
from concourse.masks import make_identity

N_NODES, N_EDGES, N_GRAPHS = 100000, 1600000, 1024
IN_LOCAL, IN_GLOBAL, HIDDEN, NUM_CLASSES = 16, 8, 128, 2
P = 128
N_CORES = 8
SLICE = N_NODES // N_CORES            # 12500
NBLK = 104                            # blocks per core (balanced, ~120/blk)
PAD_SLICE = NBLK * P                  # 13312
GBLK = N_GRAPHS // P                  # 8
UGRP = 8                              # phase-1 groups
UBLK = NBLK // UGRP                   # 13 blocks per group

CHUNKS = (26, 26, 26, 26)             # AG chunk sizes in dst blocks
CH_START = (0, 26, 52, 78)
NCHUNK = 4
RB = CHUNKS[0]                        # blocks per chunk region
REG_N = SLICE // NCHUNK               # original nodes per region (3125)
G_BLKS = 8                            # phase-2 gather group size (blocks)
NGRP = NBLK // G_BLKS                 # 13

f32 = mybir.dt.float32
bf16 = mybir.dt.bfloat16
i16 = mybir.dt.int16
np_bf16 = ml_dtypes.bfloat16

_prog_cache: dict = {}
last_run: dict = {}


# --------------------------------------------------------------------------
# device program
# --------------------------------------------------------------------------
def _build(k_used):
    """k_used: [NBLK][NCHUNK] chunk counts (static schedule, all cores)."""
    k_used = np.asarray(k_used)
    tch = k_used.sum(axis=1)                      # one-hot cols per block
    TCHMAX = int(tch.max())
    boff = np.concatenate([[0], np.cumsum(tch)])  # dstT col offset per block
    TOTCH = int(boff[-1])
    # per (group, chunk-bucket) gather call capacities (in 128-chunks)
    gcap = np.zeros((NGRP, NCHUNK), np.int64)
    for g in range(NGRP):
        gcap[g] = k_used[g * G_BLKS:(g + 1) * G_BLKS].sum(axis=0)
    GCAPM = [int(gcap[:, q].max()) for q in range(NCHUNK)]
    # idx-table col offset (in int16/16 cols) per call, group-major
    ixoff = np.zeros((NGRP, NCHUNK + 1), np.int64)
    off = 0
    for g in range(NGRP):
        for q in range(NCHUNK):
            ixoff[g][q] = off
            off += gcap[g][q] * 8        # cap*128 idxs -> /16 cols
        ixoff[g][NCHUNK] = off
    IXCOLS = int(off)
    IXGW = int(max(ixoff[g][NCHUNK] - ixoff[g][0] for g in range(NGRP)))

    nc = bacc.Bacc("TRN2", target_bir_lowering=False, debug=False,
                   num_devices=N_CORES, num_swdge_queues=4,
                   dynamic_dma_scratch_size=16384)

    def inp(name, shape, dt):
        return nc.dram_tensor(name, shape, dt, kind="ExternalInput").ap()

    xT_d = inp("xT", [IN_LOCAL, PAD_SLICE], bf16)
    gfT_d = inp("gfT", [IN_GLOBAL, N_GRAPHS], f32)
    ou_d = inp("OU", [P, PAD_SLICE], bf16)
    sel_d = inp("SEL", [P, NCHUNK * GBLK * P], bf16)
    ixe_d = inp("ixe", [P, IXCOLS], i16)
    dstT_d = inp("dstT", [P, TOTCH], bf16)
    iota_d = inp("iota", [P, TCHMAX * P], bf16)
    Wg_d = inp("Wg", [IN_GLOBAL, HIDDEN], f32)
    Wc1_d = inp("Wc1", [IN_LOCAL, HIDDEN], bf16)
    Wc2_d = inp("Wc2", [HIDDEN, HIDDEN], f32)
    Wm_d = inp("Wm", [HIDDEN, HIDDEN], bf16)
    Ws_d = inp("Ws", [HIDDEN, HIDDEN], bf16)
    Wo_d = inp("Wo", [HIDDEN, NUM_CLASSES], bf16)
    bg_d = inp("bg_c", [HIDDEN, 1], f32)
    bc_d = inp("bc_b", [P, HIDDEN], f32)
    bm_d = inp("bm_r", [1, HIDDEN], bf16)
    ones_d = inp("ones_r", [1, P], bf16)
    bs_d = inp("bs_c", [HIDDEN, 1], f32)
    bo_d = inp("bo_c", [NUM_CLASSES, 1], f32)
    idb_d = inp("idb", [P, P], bf16)
    outT_d = nc.dram_tensor("outT", [NUM_CLASSES, PAD_SLICE], f32,
                            kind="ExternalOutput").ap()

    m_sl = [nc.dram_tensor(f"m_sl{k}", [CHUNKS[k] * P, HIDDEN], bf16).ap()
            for k in range(NCHUNK)]
    m_full = [nc.dram_tensor(f"m_full{k}",
                             [N_CORES * CHUNKS[k] * P, HIDDEN], bf16,
                             addr_space="Shared").ap()
              for k in range(NCHUNK)]

    AF = mybir.ActivationFunctionType
    OP = mybir.AluOpType
    chunk_of = np.zeros(NBLK, np.int64)
    for k in range(NCHUNK):
        chunk_of[CH_START[k]:CH_START[k] + CHUNKS[k]] = k

    with tile.TileContext(nc) as tc:
        with (
            tc.tile_pool(name="const", bufs=1) as cpool,
            tc.tile_pool(name="persist", bufs=1) as ppool,
            tc.tile_pool(name="work", bufs=3) as wpool,
            tc.tile_pool(name="sbig", bufs=2) as spool,
            tc.tile_pool(name="xg", bufs=2) as xgpool,
            tc.tile_pool(name="oug", bufs=2) as oupool,
            tc.tile_pool(name="mg", bufs=3) as mgpool,
            tc.tile_pool(name="ixg", bufs=3) as ixpool,
            tc.tile_pool(name="ps_a", bufs=3, space="PSUM") as ps_a,
            tc.tile_pool(name="ps_b", bufs=3, space="PSUM") as ps_b,
            tc.tile_pool(name="ps_t", bufs=1, space="PSUM") as ps_t,
            tc.tile_pool(name="ps_o", bufs=1, space="PSUM") as ps_o,
        ):
            nc.gpsimd.load_library(mlp)

            def ctile(name, ap, shape, dt):
                t = cpool.tile(shape, dt, tag=f"c_{name}", name=f"t_{name}")
                nc.sync.dma_start(t[:], ap[:])
                return t

            Wg_t = ctile("Wg", Wg_d, [IN_GLOBAL, HIDDEN], f32)
            Wc1_t = ctile("Wc1", Wc1_d, [IN_LOCAL, HIDDEN], bf16)
            Wc2_t = ctile("Wc2", Wc2_d, [HIDDEN, HIDDEN], f32)
            Wm_t = ctile("Wm", Wm_d, [HIDDEN, HIDDEN], bf16)
            Ws_t = ctile("Ws", Ws_d, [HIDDEN, HIDDEN], bf16)
            Wo_t = ctile("Wo", Wo_d, [HIDDEN, NUM_CLASSES], bf16)
            bg_t = ctile("bg", bg_d, [HIDDEN, 1], f32)
            bc_t = ctile("bc", bc_d, [P, HIDDEN], f32)
            bm_t = ctile("bm", bm_d, [1, HIDDEN], bf16)
            ones_t = ctile("ones", ones_d, [1, P], bf16)
            bs_t = ctile("bs", bs_d, [HIDDEN, 1], f32)
            bo_t = ctile("bo", bo_d, [NUM_CLASSES, 1], f32)
            gfT_t = ctile("gfT", gfT_d, [IN_GLOBAL, N_GRAPHS], f32)
            idb_t = ctile("idb", idb_d, [P, P], bf16)
            iota_t = ctile("iota", iota_d, [P, TCHMAX * P], bf16)
            dstT_t = ctile("dstT", dstT_d, [P, TOTCH], bf16)
            sel_t = ctile("SEL", sel_d, [P, NCHUNK * GBLK * P], bf16)

            ident = cpool.tile([P, P], f32)
            make_identity(nc, ident[:])

            h0T_t = ppool.tile([HIDDEN, PAD_SLICE], bf16)   # 3.3 MB
            m16_t = ppool.tile([P, PAD_SLICE], bf16)        # 3.3 MB

            # ---------------- phase 0: global encoder ----------------
            ub8 = []
            for g in range(GBLK):
                gsl = slice(g * P, (g + 1) * P)
                ps1 = ps_b.tile([P, P], f32, tag="pb")
                nc.tensor.matmul(out=ps1[:], lhsT=Wg_t[:], rhs=gfT_t[:, gsl],
                                 start=True, stop=True)
                rT = wpool.tile([P, P], f32, tag="rT")
                nc.scalar.activation(out=rT[:], in_=ps1[:], func=AF.Relu,
                                     bias=bg_t[:, :1])
                ps2 = ps_b.tile([P, P], f32, tag="pb")
                nc.tensor.matmul(out=ps2[:], lhsT=Wc2_t[:], rhs=rT[:],
                                 start=True, stop=True)
                uT = wpool.tile([P, P], f32, tag="uT")
                nc.vector.tensor_copy(out=uT[:], in_=ps2[:])
                ps3 = ps_t.tile([P, P], f32, tag="pt")
                nc.tensor.transpose(out=ps3[:], in_=uT[:], identity=ident[:])
                ub = ppool.tile([P, HIDDEN], bf16, tag=f"ub{g}",
                                name=f"ub{g}")
                nc.vector.tensor_tensor(out=ub[:], in0=ps3[:], in1=bc_t[:],
                                        op=OP.add)
                ub8.append(ub)

            # per-region 128-graph windows uR[r] = SEL_r^T @ u
            sel_v = sel_t[:].rearrange("p (r g w) -> p r g w", r=NCHUNK,
                                       g=GBLK)
            uR_t = []
            for r in range(NCHUNK):
                psw = ps_t.tile([P, HIDDEN], f32, tag="pt")
                for gt in range(GBLK):
                    nc.tensor.matmul(out=psw[:], lhsT=sel_v[:, r, gt, :],
                                     rhs=ub8[gt][:], start=(gt == 0),
                                     stop=(gt == GBLK - 1))
                t = ppool.tile([P, HIDDEN], bf16, tag=f"uR{r}",
                               name=f"uR{r}")
                nc.scalar.activation(out=t[:], in_=psw[:], func=AF.Identity)
                uR_t.append(t)

            # ---------------- phase 1: h0 / m on own slice ----------------
            for uc in range(UGRP):
                csl = slice(uc * UBLK * P, (uc + 1) * UBLK * P)
                xgf = xgpool.tile([IN_LOCAL, UBLK * P], bf16, tag="xg")
                nc.sync.dma_start(xgf[:], xT_d[:, csl])
                oug = oupool.tile([P, UBLK * P], bf16, tag="ou")
                nc.sync.dma_start(oug[:], ou_d[:, csl])
                for j in range(UBLK):
                    b = uc * UBLK + j
                    bsl = slice(b * P, (b + 1) * P)
                    jsl = slice(j * P, (j + 1) * P)
                    psh = ps_b.tile([P, P], f32, tag="pb")
                    nc.tensor.matmul(out=psh[:], lhsT=Wc1_t[:],
                                     rhs=xgf[:, jsl], start=True, stop=False)
                    nc.tensor.matmul(out=psh[:],
                                     lhsT=uR_t[int(chunk_of[b])][:],
                                     rhs=oug[:, jsl], start=False, stop=True)
                    nc.scalar.activation(out=h0T_t[:, bsl], in_=psh[:],
                                         func=AF.Relu)
                    psm = ps_b.tile([P, P], f32, tag="pb")
                    nc.tensor.matmul(out=psm[:], lhsT=h0T_t[:, bsl],
                                     rhs=Wm_t[:], start=True, stop=False)
                    nc.tensor.matmul(out=psm[:], lhsT=ones_t[:],
                                     rhs=bm_t[:], start=False, stop=True)
                    nc.scalar.activation(out=m16_t[:, bsl], in_=psm[:],
                                         func=AF.Relu)
                    k = int(chunk_of[b])
                    rb = b - CH_START[k]
                    nc.sync.dma_start(m_sl[k][rb * P:(rb + 1) * P, :],
                                      m16_t[:, bsl])
                    # trigger the chunk's AllGather as soon as its last
                    # block is written (overlaps rest of phase 1)
                    if b == CH_START[k] + CHUNKS[k] - 1:
                        nc.gpsimd.collective_compute(
                            "AllGather", OP.bypass,
                            replica_groups=[list(range(N_CORES))],
                            ins=[m_sl[k][:]], outs=[m_full[k][:]])

            # ---------------- phase 2: scatter-add + update + readout ------
            iota_v = iota_t[:].rearrange("p (k f) -> p k f", k=TCHMAX)
            mg_tiles = {}

            def emit_gathers(g):
                gw = int(ixoff[g][NCHUNK] - ixoff[g][0])
                ixg = ixpool.tile([P, IXGW], i16, tag="ixg")
                nc.sync.dma_start(
                    ixg[:, :gw],
                    ixe_d[:, int(ixoff[g][0]):int(ixoff[g][NCHUNK])])
                for q in range(NCHUNK):
                    # ONE call per (group, bucket); queue q only ever waits
                    # on AllGather q so the 4 queues stream independently
                    cap = int(gcap[g][q])
                    c0 = int(ixoff[g][q] - ixoff[g][0])
                    t = mgpool.tile([P, GCAPM[q], HIDDEN], bf16,
                                    tag=f"mg{q}", name=f"mg{q}_{g}")
                    nc.gpsimd.dma_gather(
                        t[:, :cap, :], m_full[q][:],
                        ixg[:, c0:c0 + cap * 8],
                        cap * P, cap * P, HIDDEN,
                        single_packet=False, queue_num=q)
                    mg_tiles[(g, q)] = t

            emit_gathers(0)
            emit_gathers(1)
            for g in range(NGRP):
                if g + 2 < NGRP:
                    emit_gathers(g + 2)
                # column offsets of each block's tiles within the group call
                coltab = np.zeros((G_BLKS, NCHUNK), np.int64)
                run = np.zeros(NCHUNK, np.int64)
                for j in range(G_BLKS):
                    coltab[j] = run
                    run += k_used[g * G_BLKS + j]
                for j in range(G_BLKS):
                    b = g * G_BLKS + j
                    bsl = slice(b * P, (b + 1) * P)
                    tchb = int(k_used[b].sum())
                    S = spool.tile([P, TCHMAX, P], bf16, tag="S")
                    nc.vector.tensor_tensor(
                        out=S[:, :tchb, :],
                        in0=dstT_t[:, int(boff[b]):int(boff[b + 1])]
                            .to_broadcast([P, tchb, P]),
                        in1=iota_v[:, :tchb, :], op=OP.is_equal)
                    pa = ps_a.tile([HIDDEN, P], f32, tag="pa")
                    # self loops: aggrT += m_block^T
                    nc.tensor.matmul(out=pa[:], lhsT=m16_t[:, bsl],
                                     rhs=idb_t[:], start=True, stop=False)
                    col = 0
                    for q in range(NCHUNK):
                        mg = mg_tiles[(g, q)]
                        for c in range(int(k_used[b][q])):
                            nc.tensor.matmul(
                                out=pa[:],
                                lhsT=mg[:, int(coltab[j][q]) + c, :],
                                rhs=S[:, col, :], start=False, stop=False)
                            col += 1
                    nc.tensor.matmul(out=pa[:], lhsT=Ws_t[:],
                                     rhs=h0T_t[:, bsl], start=False, stop=True)
                    hT = wpool.tile([HIDDEN, P], bf16, tag="hT")
                    nc.scalar.activation(out=hT[:], in_=pa[:], func=AF.Relu,
                                         bias=bs_t[:, :1])
                    po = ps_o.tile([NUM_CLASSES, P], f32, tag="po")
                    nc.tensor.matmul(out=po[:], lhsT=Wo_t[:], rhs=hT[:],
                                     start=True, stop=True)
                    ob = wpool.tile([NUM_CLASSES, P], f32, tag="ob")
                    nc.scalar.activation(out=ob[:], in_=po[:],
                                         func=AF.Identity, bias=bo_t[:, :1])
                    nc.sync.dma_start(outT_d[:, b * P:(b + 1) * P], ob[:])
                for q in range(NCHUNK):
                    del mg_tiles[(g, q)]

    nc.compile()
    return nc


# --------------------------------------------------------------------------
# host side
# --------------------------------------------------------------------------
def _wrap16(ix):
    """dma_gather int16 index layout: [16, n/16] wrapped, tiled to 128 parts."""
    return np.tile(ix.reshape(-1, 16).T, (8, 1))


def _balance(dst_all, src_all):
    """Assign each core's nodes to NBLK blocks (<=128 each) so that every
    (block, src-chunk-region) in-degree bucket stays <= 4*128.  A node's
    chunk region (for its outgoing edges) is its original quarter of the
    core slice, so regions are fixed before balancing and the per-core
    problems decouple."""
    region_of = (np.arange(N_NODES) % SLICE) // REG_N
    q_src = region_of[src_all]
    core_of = dst_all // SLICE
    pos_of = np.zeros((N_CORES, SLICE), np.int64)
    for c in range(N_CORES):
        sel = core_of == c
        d_loc = dst_all[sel] - c * SLICE
        deg = np.zeros((SLICE, NCHUNK), np.int32)
        np.add.at(deg, (d_loc, q_src[sel]), 1)
        for ch in range(NCHUNK):
            nodes = np.arange(ch * REG_N, (ch + 1) * REG_N)
            dv = deg[nodes]
            order = np.argsort(-dv.sum(axis=1), kind="stable")
            loads = np.zeros((RB, NCHUNK), np.int64)
            cnts = np.zeros(RB, np.int64)
            for n in order:
                score = (loads + dv[n]).max(axis=1) * 1024 + cnts
                score[cnts >= P] = 1 << 60
                bb = int(np.argmin(score))
                pos_of[c][nodes[n]] = (ch * RB + bb) * P + cnts[bb]
                loads[bb] += dv[n]
                cnts[bb] += 1
    return pos_of


def _preprocess(inputs):
    x = np.asarray(inputs["x"], dtype=np.float32)
    ei = np.asarray(inputs["edge_index"]).astype(np.int64)
    batch = np.asarray(inputs["batch"]).astype(np.int64)
    gf = np.asarray(inputs["global_feat"], dtype=np.float32)
    W = {k: np.ascontiguousarray(np.asarray(inputs[k], dtype=np.float32))
         for k in ("Wg", "bg", "Wc", "bc", "Wm", "bm", "Ws", "bs", "Wo", "bo")}

    src_all, dst_all = ei[0], ei[1]
    pos_of = _balance(dst_all, src_all)
    # src -> (chunk bucket, row within that chunk's m_full tensor)
    s_core = src_all // SLICE
    s_pos = pos_of[s_core, src_all % SLICE]
    s_blk = s_pos // P
    s_ch = s_blk // RB
    s_row = (s_core * (RB * P) + (s_blk - s_ch * RB) * P
             + s_pos % P).astype(np.int16)
    core_of = dst_all // SLICE

    per_core = []
    counts = np.zeros((N_CORES, NBLK, NCHUNK), np.int64)
    for c in range(N_CORES):
        sel = np.nonzero(core_of == c)[0]
        d_pos = pos_of[c][dst_all[sel] - c * SLICE]
        blk = d_pos // P
        q = s_ch[sel]
        key = blk * NCHUNK + q
        order = np.argsort(key, kind="stable")
        sel, key = sel[order], key[order]
        cnt = np.bincount(key, minlength=NBLK * NCHUNK).reshape(NBLK, NCHUNK)
        counts[c] = cnt
        per_core.append((key, (d_pos[order] % P).astype(np.float32),
                         s_row[sel], cnt))

    k_used = (-(-counts.max(axis=0) // P)).astype(np.int64)  # [NBLK, NCHUNK]
    tch = k_used.sum(axis=1)
    boff = np.concatenate([[0], np.cumsum(tch)])
    TOTCH = int(boff[-1])
    TCHMAX = int(tch.max())

    # static call layout: per (group, bucket) capacity and flat offsets
    gcap = np.zeros((NGRP, NCHUNK), np.int64)
    for g in range(NGRP):
        gcap[g] = k_used[g * G_BLKS:(g + 1) * G_BLKS].sum(axis=0)
    # flat idx-slot offset of bucket (b, q) inside the concatenated calls
    bucket_off = np.zeros((NBLK, NCHUNK), np.int64)
    off = 0
    for g in range(NGRP):
        for q in range(NCHUNK):
            for j in range(G_BLKS):
                b = g * G_BLKS + j
                bucket_off[b][q] = off
                off += k_used[b][q] * P
    TOTSLOT = int(off)

    iota_np = np.tile(np.arange(P, dtype=np.float32).astype(np_bf16),
                      (P, TCHMAX))
    shared = {
        "gfT": np.ascontiguousarray(gf.T),
        "iota": iota_np,
        "idb": np.eye(P, dtype=np.float32).astype(np_bf16),
        "Wg": W["Wg"],
        "Wc1": np.ascontiguousarray(W["Wc"][:IN_LOCAL]).astype(np_bf16),
        "Wc2": np.ascontiguousarray(W["Wc"][IN_LOCAL:]),
        "Wm": W["Wm"].astype(np_bf16),
        "Ws": W["Ws"].astype(np_bf16),
        "Wo": W["Wo"].astype(np_bf16),
        "bg_c": W["bg"].reshape(HIDDEN, 1),
        "bc_b": np.tile(W["bc"], (P, 1)),
        "bm_r": W["bm"].reshape(1, HIDDEN).astype(np_bf16),
        "ones_r": np.ones((1, P), np.float32).astype(np_bf16),
        "bs_c": W["bs"].reshape(HIDDEN, 1),
        "bo_c": W["bo"].reshape(NUM_CLASSES, 1),
    }

    in_maps = []
    for c in range(N_CORES):
        key, d128, l16, cnt = per_core[c]
        ix_pad = np.zeros(TOTSLOT, np.int16)
        dst_pad = np.full(TOTCH * P, -1.0, np.float32)
        flat_cnt = cnt.reshape(-1)
        cum = np.cumsum(flat_cnt) - flat_cnt
        within = np.arange(len(key)) - np.repeat(cum, flat_cnt)
        # gather-call slot (group-major) and dstT slot (block-major)
        pos_ix = np.repeat(bucket_off.reshape(-1), flat_cnt) + within
        dst_off = (boff[:-1][:, None]
                   + np.concatenate([np.zeros((NBLK, 1), np.int64),
                                     np.cumsum(k_used, axis=1)[:, :-1]],
                                    axis=1)) * P
        pos_dst = np.repeat(dst_off.reshape(-1), flat_cnt) + within
        ix_pad[pos_ix] = l16
        dst_pad[pos_dst] = d128
        ixe = np.concatenate(
            [_wrap16(ix_pad[bucket_off[g * G_BLKS][q]:
                            bucket_off[g * G_BLKS][q] + gcap[g][q] * P])
             for g in range(NGRP) for q in range(NCHUNK)], axis=1)
        dstT = np.ascontiguousarray(
            dst_pad.reshape(-1, P).T).astype(np_bf16)

        # u[batch] one-hots: per region a 128-graph window (base per core,
        # encoded in SEL), per node-slot a one-hot of batch - base.
        bpad = np.zeros(PAD_SLICE, np.int64)
        bpad[pos_of[c]] = batch[c * SLICE:(c + 1) * SLICE]
        real = np.zeros(PAD_SLICE, bool)
        real[pos_of[c]] = True
        ou = np.zeros((P, PAD_SLICE), np.int8)
        sel_oh = np.zeros((P, NCHUNK, GBLK, P), np.int8)
        for r in range(NCHUNK):
            n0 = c * SLICE + r * REG_N
            base = min(int(batch[n0]), N_GRAPHS - P)
            ssl = slice(r * RB * P, (r + 1) * RB * P)
            rel = bpad[ssl] - base
            rr = real[ssl]
            assert (rel[rr] >= 0).all() and (rel[rr] < P).all(), \
                f"core {c} region {r}: batch window exceeds {P} graphs"
            cols = np.arange(ssl.start, ssl.stop)[rr]
            ou[rel[rr], cols] = 1
            # SEL[grow, r, gt, w] = 1 iff global graph gt*128+grow == base+w
            w = np.arange(P)
            gabs = base + w
            sel_oh[gabs % P, r, gabs // P, w] = 1

        xT = np.zeros((IN_LOCAL, PAD_SLICE), np_bf16)
        xT[:, pos_of[c]] = x[c * SLICE:(c + 1) * SLICE].T.astype(np_bf16)

        m = dict(shared)
        m.update({"xT": xT, "OU": ou.astype(np_bf16),
                  "SEL": sel_oh.reshape(P, NCHUNK * GBLK * P).astype(np_bf16),
                  "ixe": ixe, "dstT": dstT})
        in_maps.append(m)
    return k_used, pos_of, in_maps


# --------------------------------------------------------------------------
# profiling hook (NTFF via the axon PJRT .so; absent module in this image)
# --------------------------------------------------------------------------
def _profile_hook():
    so = "/opt/axon/libaxon_pjrt.so"
    if not os.path.exists(so):
        return None
    lib = ctypes.CDLL(so)
    if not hasattr(lib, "axon_start_nrt_profile"):
        return None
    lib.axon_start_nrt_profile.argtypes = [ctypes.POINTER(ctypes.c_int64),
                                           ctypes.c_size_t]
    lib.axon_start_nrt_profile.restype = ctypes.c_int64
    lib.axon_stop_nrt_profile.argtypes = [ctypes.c_char_p]
    lib.axon_stop_nrt_profile.restype = ctypes.c_int64

    @contextlib.contextmanager
    def hook(output_dir, device_ids):
        import jax
        jax.devices()
        if device_ids:
            ids = (ctypes.c_int64 * len(device_ids))(*device_ids)
            rc = lib.axon_start_nrt_profile(ids, len(device_ids))
        else:
            rc = lib.axon_start_nrt_profile(None, 0)
        if rc != 0:
            raise RuntimeError(f"axon_start_nrt_profile rc={rc}")
        try:
            yield
        finally:
            n = lib.axon_stop_nrt_profile(str(output_dir).encode())
            print(f"profile: {n} file(s) written to {output_dir}",
                  file=sys.stderr)

    return hook


def _run(nc, in_maps):
    from concourse import bass2jax
    trace_dir = os.environ.get("GNN_TRACE_DIR", "")
    if not trace_dir:
        return bass2jax.run_bass_via_pjrt(nc, in_maps, n_cores=N_CORES)
    hook = _profile_hook()
    if hook is None:
        return bass2jax.run_bass_via_pjrt(nc, in_maps, n_cores=N_CORES)
    import time as _time
    trace_dir = os.path.join(trace_dir, f"run_{int(_time.time()*1000)}")
    os.makedirs(trace_dir, exist_ok=True)
    last_run["trace_dir"] = trace_dir
    trace_cores = [int(t) for t in
                   os.environ.get("GNN_TRACE_CORES", "0").split(",")]
    with hook(trace_dir, trace_cores):
        results = bass2jax.run_bass_via_pjrt(nc, in_maps, n_cores=N_CORES)
    try:
        from concourse._compat import FishPath
        import gauge.profiler as gprof
        profile = gprof.Profile(
            profile_path=FishPath(trace_dir), kernel_dev_mode=True,
            profile_on_exit=False, bass_kernel=nc.m,
            offline_processing=True, fname="*_body*")
        profile.convert_ntffs_to_json(tuple(trace_cores))
        j = profile.load_json(trace_cores[0])
        last_run["summary"] = j["summary"][0] if j else None
        last_run["exec_time_ns"] = (
            int(j["summary"][0]["total_time"] * 1e9) if j else None)
        last_run["profile_json"] = str(profile.json_path(trace_cores[0]))
    except Exception as e:  # profiling must never break the run
        print(f"profile post-processing failed: {e}", file=sys.stderr)
    return results


def kernel(**inputs) -> np.ndarray:
    k_used, pos_of, in_maps = _preprocess(inputs)
    key = k_used.tobytes()
    nc = _prog_cache.get(key)
    if nc is None:
        nc = _build(k_used)
        _prog_cache[key] = nc
    last_run.clear()
    results = _run(nc, in_maps)
    out = np.empty((N_NODES, NUM_CLASSES), np.float32)
    for c in range(N_CORES):
        out[c * SLICE:(c + 1) * SLICE] = results[c]["outT"].T[pos_of[c]]
    return out


# revision 20
# speedup vs baseline: 1.5055x; 1.2930x over previous
"""Trainium2 Bass kernel for nn_BaselineGNN (GNN message passing).

Strategy (8 NeuronCores, SPMD), v3:
  - Node-partition: core c owns dst nodes [c*12500, (c+1)*12500), 104 blocks
    of 128 (PAD_SLICE=13312).
  - All PE-path tensors bf16 (fp32 matmuls run multiple passes on the PE).
  - Phase 1 computes h0/m with NO gpsimd work: u = relu(gf@Wg+bg)@Wc2+bc
    lives in SBUF as 8 [128-graph, H] tiles; per region (quarter core
    slice, < 128 distinct sorted graph ids) a per-core one-hot input SEL
    selects a 128-graph window uR, and per block the u[batch] term is ONE
    matmul uR @ OU with host-built one-hots.  ReLUs run on the Scalar
    engine; bm enters psm via a rank-1 ones x bm matmul.
  - m AllGather split into 4 chunk collectives ([26,26,26,26] blocks) that
    fire as soon as each chunk's m blocks are written (overlaps phase 1).
  - Phase 2: per (group of 8 dst blocks, chunk bucket q) ONE dma_gather of
    ~4096 rows on queue q.  queue q only ever waits on AllGather q, so the
    4 SWDGE queues stream independently with no head-of-line blocking.
    Padding slots use idx 0 (a valid row) with dst one-hot column -1 -> 0.
  - Scatter-add per block: aggrT accumulates in PSUM as
      m_blk^T (self loops, identity matmul)
      + sum_chunks mg[128e,H]^T @ onehot(dst)[128e,128d]
      + Ws^T @ h0T
    then hT = relu(aggrT+bs) (scalar engine), outT = Wo^T@hT+bo into an
    SBUF accumulator, DMA'd once at the end.
"""
import contextlib
import ctypes
import os
import sys

sys.path.insert(0, "/opt/trn_rl_repo")

import numpy as np
import ml_dtypes

import concourse.bass as bass
import concourse.bacc as bacc
import concourse.tile as tile
from concourse import mybir
from concourse.library_config import mlp
from concourse.masks import make_identity

N_NODES, N_EDGES, N_GRAPHS = 100000, 1600000, 1024
IN_LOCAL, IN_GLOBAL, HIDDEN, NUM_CLASSES = 16, 8, 128, 2
P = 128
N_CORES = 8
SLICE = N_NODES // N_CORES            # 12500
NBLK = 104                            # blocks per core (balanced, ~120/blk)
PAD_SLICE = NBLK * P                  # 13312
GBLK = N_GRAPHS // P                  # 8
UGRP = 8                              # phase-1 groups
UBLK = NBLK // UGRP                   # 13 blocks per group

CHUNKS = (26, 26, 26, 26)             # AG chunk sizes in dst blocks
CH_START = (0, 26, 52, 78)
NCHUNK = 4
RB = CHUNKS[0]                        # blocks per chunk region
REG_N = SLICE // NCHUNK               # original nodes per region (3125)
G_BLKS = 8                            # phase-2 gather group size (blocks)
NGRP = NBLK // G_BLKS                 # 13

f32 = mybir.dt.float32
bf16 = mybir.dt.bfloat16
fp8 = mybir.dt.float8e4
i16 = mybir.dt.int16
np_bf16 = ml_dtypes.bfloat16

_prog_cache: dict = {}
last_run: dict = {}


# --------------------------------------------------------------------------
# device program
# --------------------------------------------------------------------------
def _build(k_used):
    """k_used: [NBLK][NCHUNK] chunk counts (static schedule, all cores)."""
    k_used = np.asarray(k_used)
    tch = k_used.sum(axis=1)                      # one-hot cols per block
    TCHMAX = int(tch.max())
    boff = np.concatenate([[0], np.cumsum(tch)])  # dstT col offset per block
    TOTCH = int(boff[-1])
    # per (group, chunk-bucket) gather call capacities (in 128-chunks)
    gcap = np.zeros((NGRP, NCHUNK), np.int64)
    for g in range(NGRP):
        gcap[g] = k_used[g * G_BLKS:(g + 1) * G_BLKS].sum(axis=0)
    GCAPM = [int(gcap[:, q].max()) for q in range(NCHUNK)]
    # idx-table col offset (in int16/16 cols) per call, group-major
    ixoff = np.zeros((NGRP, NCHUNK + 1), np.int64)
    off = 0
    for g in range(NGRP):
        for q in range(NCHUNK):
            ixoff[g][q] = off
            off += gcap[g][q] * 8        # cap*128 idxs -> /16 cols
        ixoff[g][NCHUNK] = off
    IXCOLS = int(off)
    IXGW = int(max(ixoff[g][NCHUNK] - ixoff[g][0] for g in range(NGRP)))

    nc = bacc.Bacc("TRN2", target_bir_lowering=False, debug=False,
                   num_devices=N_CORES, num_swdge_queues=4,
                   dynamic_dma_scratch_size=16384)

    def inp(name, shape, dt):
        return nc.dram_tensor(name, shape, dt, kind="ExternalInput").ap()

    xT_d = inp("xT", [IN_LOCAL, PAD_SLICE], bf16)
    gfT_d = inp("gfT", [IN_GLOBAL, N_GRAPHS], f32)
    ou_d = inp("OU", [P, PAD_SLICE], bf16)
    sel_d = inp("SEL", [P, NCHUNK * GBLK * P], bf16)
    ixe_d = inp("ixe", [P, IXCOLS], i16)
    dstT_d = inp("dstT", [P, TOTCH], bf16)
    iota_d = inp("iota", [P, TCHMAX * P], bf16)
    Wg_d = inp("Wg", [IN_GLOBAL, HIDDEN], f32)
    Wc1_d = inp("Wc1", [IN_LOCAL, HIDDEN], bf16)
    Wc2_d = inp("Wc2", [HIDDEN, HIDDEN], f32)
    Wm_d = inp("Wm", [HIDDEN, HIDDEN], bf16)
    Ws_d = inp("Ws", [HIDDEN, HIDDEN], bf16)
    Wo_d = inp("Wo", [HIDDEN, NUM_CLASSES], bf16)
    bg_d = inp("bg_c", [HIDDEN, 1], f32)
    bc_d = inp("bc_b", [P, HIDDEN], f32)
    bm_d = inp("bm_r", [1, HIDDEN], bf16)
    ones_d = inp("ones_r", [1, P], bf16)
    bs_d = inp("bs_c", [HIDDEN, 1], f32)
    bo_d = inp("bo_c", [NUM_CLASSES, 1], f32)
    idb_d = inp("idb", [P, P], bf16)
    outT_d = nc.dram_tensor("outT", [NUM_CLASSES, PAD_SLICE], f32,
                            kind="ExternalOutput").ap()

    # fp8 message rows stored duplicated (row r = (m8[r], m8[r]), 256B) so
    # a 256B gather descriptor is legal for any row; matmuls read the first
    # 128B half only
    m_sl = [nc.dram_tensor(f"m_sl{k}", [CHUNKS[k] * P, 2 * HIDDEN],
                           fp8).ap() for k in range(NCHUNK)]
    m_full = [nc.dram_tensor(f"m_full{k}",
                             [N_CORES * CHUNKS[k] * P, 2 * HIDDEN], fp8,
                             addr_space="Shared").ap()
              for k in range(NCHUNK)]

    AF = mybir.ActivationFunctionType
    OP = mybir.AluOpType
    chunk_of = np.zeros(NBLK, np.int64)
    for k in range(NCHUNK):
        chunk_of[CH_START[k]:CH_START[k] + CHUNKS[k]] = k

    with tile.TileContext(nc) as tc:
        with (
            tc.tile_pool(name="const", bufs=1) as cpool,
            tc.tile_pool(name="persist", bufs=1) as ppool,
            tc.tile_pool(name="work", bufs=3) as wpool,
            tc.tile_pool(name="sbig", bufs=2) as spool,
            tc.tile_pool(name="xg", bufs=2) as xgpool,
            tc.tile_pool(name="oug", bufs=2) as oupool,
            tc.tile_pool(name="mg", bufs=3) as mgpool,
            tc.tile_pool(name="ixg", bufs=3) as ixpool,
            tc.tile_pool(name="ps_a", bufs=3, space="PSUM") as ps_a,
            tc.tile_pool(name="ps_b", bufs=3, space="PSUM") as ps_b,
            tc.tile_pool(name="ps_t", bufs=1, space="PSUM") as ps_t,
            tc.tile_pool(name="ps_o", bufs=1, space="PSUM") as ps_o,
        ):
            nc.gpsimd.load_library(mlp)

            def ctile(name, ap, shape, dt):
                t = cpool.tile(shape, dt, tag=f"c_{name}", name=f"t_{name}")
                nc.sync.dma_start(t[:], ap[:])
                return t

            Wg_t = ctile("Wg", Wg_d, [IN_GLOBAL, HIDDEN], f32)
            Wc1_t = ctile("Wc1", Wc1_d, [IN_LOCAL, HIDDEN], bf16)
            Wc2_t = ctile("Wc2", Wc2_d, [HIDDEN, HIDDEN], f32)
            Wm_t = ctile("Wm", Wm_d, [HIDDEN, HIDDEN], bf16)
            Ws_t = ctile("Ws", Ws_d, [HIDDEN, HIDDEN], bf16)
            Wo_t = ctile("Wo", Wo_d, [HIDDEN, NUM_CLASSES], bf16)
            bg_t = ctile("bg", bg_d, [HIDDEN, 1], f32)
            bc_t = ctile("bc", bc_d, [P, HIDDEN], f32)
            bm_t = ctile("bm", bm_d, [1, HIDDEN], bf16)
            ones_t = ctile("ones", ones_d, [1, P], bf16)
            bs_t = ctile("bs", bs_d, [HIDDEN, 1], f32)
            bo_t = ctile("bo", bo_d, [NUM_CLASSES, 1], f32)
            gfT_t = ctile("gfT", gfT_d, [IN_GLOBAL, N_GRAPHS], f32)
            idb_t = ctile("idb", idb_d, [P, P], bf16)
            iota_t = ctile("iota", iota_d, [P, TCHMAX * P], bf16)
            dstT_t = ctile("dstT", dstT_d, [P, TOTCH], bf16)
            sel_t = ctile("SEL", sel_d, [P, NCHUNK * GBLK * P], bf16)

            ident = cpool.tile([P, P], f32)
            make_identity(nc, ident[:])

            h0T_t = ppool.tile([HIDDEN, PAD_SLICE], bf16)   # 3.3 MB
            m16_t = ppool.tile([P, PAD_SLICE], bf16)        # 3.3 MB

            # ---------------- phase 0: global encoder ----------------
            ub8 = []
            for g in range(GBLK):
                gsl = slice(g * P, (g + 1) * P)
                ps1 = ps_b.tile([P, P], f32, tag="pb")
                nc.tensor.matmul(out=ps1[:], lhsT=Wg_t[:], rhs=gfT_t[:, gsl],
                                 start=True, stop=True)
                rT = wpool.tile([P, P], f32, tag="rT")
                nc.scalar.activation(out=rT[:], in_=ps1[:], func=AF.Relu,
                                     bias=bg_t[:, :1])
                ps2 = ps_b.tile([P, P], f32, tag="pb")
                nc.tensor.matmul(out=ps2[:], lhsT=Wc2_t[:], rhs=rT[:],
                                 start=True, stop=True)
                uT = wpool.tile([P, P], f32, tag="uT")
                nc.vector.tensor_copy(out=uT[:], in_=ps2[:])
                ps3 = ps_t.tile([P, P], f32, tag="pt")
                nc.tensor.transpose(out=ps3[:], in_=uT[:], identity=ident[:])
                ub = ppool.tile([P, HIDDEN], bf16, tag=f"ub{g}",
                                name=f"ub{g}")
                nc.vector.tensor_tensor(out=ub[:], in0=ps3[:], in1=bc_t[:],
                                        op=OP.add)
                ub8.append(ub)

            # per-region 128-graph windows uR[r] = SEL_r^T @ u
            sel_v = sel_t[:].rearrange("p (r g w) -> p r g w", r=NCHUNK,
                                       g=GBLK)
            uR_t = []
            for r in range(NCHUNK):
                psw = ps_t.tile([P, HIDDEN], f32, tag="pt")
                for gt in range(GBLK):
                    nc.tensor.matmul(out=psw[:], lhsT=sel_v[:, r, gt, :],
                                     rhs=ub8[gt][:], start=(gt == 0),
                                     stop=(gt == GBLK - 1))
                t = ppool.tile([P, HIDDEN], bf16, tag=f"uR{r}",
                               name=f"uR{r}")
                nc.scalar.activation(out=t[:], in_=psw[:], func=AF.Identity)
                uR_t.append(t)

            # ---------------- phase 1: h0 / m on own slice ----------------
            for uc in range(UGRP):
                csl = slice(uc * UBLK * P, (uc + 1) * UBLK * P)
                xgf = xgpool.tile([IN_LOCAL, UBLK * P], bf16, tag="xg")
                nc.sync.dma_start(xgf[:], xT_d[:, csl])
                oug = oupool.tile([P, UBLK * P], bf16, tag="ou")
                nc.sync.dma_start(oug[:], ou_d[:, csl])
                for j in range(UBLK):
                    b = uc * UBLK + j
                    bsl = slice(b * P, (b + 1) * P)
                    jsl = slice(j * P, (j + 1) * P)
                    psh = ps_b.tile([P, P], f32, tag="pb")
                    nc.tensor.matmul(out=psh[:], lhsT=Wc1_t[:],
                                     rhs=xgf[:, jsl], start=True, stop=False)
                    nc.tensor.matmul(out=psh[:],
                                     lhsT=uR_t[int(chunk_of[b])][:],
                                     rhs=oug[:, jsl], start=False, stop=True)
                    nc.scalar.activation(out=h0T_t[:, bsl], in_=psh[:],
                                         func=AF.Relu)
                    psm = ps_a.tile([P, P], f32, tag="pa")
                    nc.tensor.matmul(out=psm[:], lhsT=h0T_t[:, bsl],
                                     rhs=Wm_t[:], start=True, stop=False)
                    nc.tensor.matmul(out=psm[:], lhsT=ones_t[:],
                                     rhs=bm_t[:], start=False, stop=True)
                    nc.scalar.activation(out=m16_t[:, bsl], in_=psm[:],
                                         func=AF.Relu)
                    m8d = wpool.tile([P, 2 * P], fp8, tag="m8")
                    nc.vector.tensor_scalar_max(out=m8d[:, 0:P], in0=psm[:],
                                                scalar1=0.0)
                    nc.vector.tensor_scalar_max(out=m8d[:, P:2 * P],
                                                in0=psm[:], scalar1=0.0)
                    k = int(chunk_of[b])
                    rb = b - CH_START[k]
                    nc.sync.dma_start(m_sl[k][rb * P:(rb + 1) * P, :],
                                      m8d[:])
                    # trigger the chunk's AllGather as soon as its last
                    # block is written (overlaps rest of phase 1)
                    if b == CH_START[k] + CHUNKS[k] - 1:
                        nc.gpsimd.collective_compute(
                            "AllGather", OP.bypass,
                            replica_groups=[list(range(N_CORES))],
                            ins=[m_sl[k][:]], outs=[m_full[k][:]])

            # ---------------- phase 2: scatter-add + update + readout ------
            iota_v = iota_t[:].rearrange("p (k f) -> p k f", k=TCHMAX)
            mg_tiles = {}

            def emit_gathers(g):
                gw = int(ixoff[g][NCHUNK] - ixoff[g][0])
                ixg = ixpool.tile([P, IXGW], i16, tag="ixg")
                nc.sync.dma_start(
                    ixg[:, :gw],
                    ixe_d[:, int(ixoff[g][0]):int(ixoff[g][NCHUNK])])
                for q in range(NCHUNK):
                    # ONE call per (group, bucket); queue q only ever waits
                    # on AllGather q so the 4 queues stream independently
                    cap = int(gcap[g][q])
                    c0 = int(ixoff[g][q] - ixoff[g][0])
                    t = mgpool.tile([P, GCAPM[q], 2 * HIDDEN], fp8,
                                    tag=f"mg{q}", name=f"mg{q}_{g}")
                    nc.gpsimd.dma_gather(
                        t[:, :cap, :], m_full[q][:],
                        ixg[:, c0:c0 + cap * 8],
                        cap * P, cap * P, 2 * HIDDEN,
                        single_packet=False, queue_num=q)
                    mg_tiles[(g, q)] = t

            emit_gathers(0)
            emit_gathers(1)
            for g in range(NGRP):
                if g + 2 < NGRP:
                    emit_gathers(g + 2)
                # column offsets of each block's tiles within the group call
                coltab = np.zeros((G_BLKS, NCHUNK), np.int64)
                run = np.zeros(NCHUNK, np.int64)
                for j in range(G_BLKS):
                    coltab[j] = run
                    run += k_used[g * G_BLKS + j]
                for j in range(G_BLKS):
                    b = g * G_BLKS + j
                    bsl = slice(b * P, (b + 1) * P)
                    tchb = int(k_used[b].sum())
                    S = spool.tile([P, TCHMAX, P], fp8, tag="S")
                    nc.vector.tensor_tensor(
                        out=S[:, :tchb, :],
                        in0=dstT_t[:, int(boff[b]):int(boff[b + 1])]
                            .to_broadcast([P, tchb, P]),
                        in1=iota_v[:, :tchb, :], op=OP.is_equal)
                    pa = ps_a.tile([HIDDEN, P], f32, tag="pa")
                    # self loops: aggrT += m_block^T
                    nc.tensor.matmul(out=pa[:], lhsT=m16_t[:, bsl],
                                     rhs=idb_t[:], start=True, stop=False)
                    col = 0
                    for q in range(NCHUNK):
                        mg = mg_tiles[(g, q)]
                        kq = int(k_used[b][q])
                        for c2 in range(kq // 2):
                            cc = int(coltab[j][q]) + 2 * c2
                            # DoubleRow fp8: 256 edges per matmul; each
                            # gathered 256B col is (m8[r], m8[r]) and only
                            # the first 128B half is read as lhsT
                            nc.tensor.matmul(
                                out=pa[:],
                                lhsT=mg[:, cc:cc + 2, 0:HIDDEN],
                                rhs=S[:, col:col + 2, :],
                                perf_mode=mybir.MatmulPerfMode.DoubleRow,
                                start=False, stop=False)
                            col += 2
                    nc.tensor.matmul(out=pa[:], lhsT=Ws_t[:],
                                     rhs=h0T_t[:, bsl], start=False, stop=True)
                    hT = wpool.tile([HIDDEN, P], bf16, tag="hT")
                    nc.scalar.activation(out=hT[:], in_=pa[:], func=AF.Relu,
                                         bias=bs_t[:, :1])
                    po = ps_o.tile([NUM_CLASSES, P], f32, tag="po")
                    nc.tensor.matmul(out=po[:], lhsT=Wo_t[:], rhs=hT[:],
                                     start=True, stop=True)
                    ob = wpool.tile([NUM_CLASSES, P], f32, tag="ob")
                    nc.scalar.activation(out=ob[:], in_=po[:],
                                         func=AF.Identity, bias=bo_t[:, :1])
                    nc.sync.dma_start(outT_d[:, b * P:(b + 1) * P], ob[:])
                for q in range(NCHUNK):
                    del mg_tiles[(g, q)]

    nc.compile()
    return nc


# --------------------------------------------------------------------------
# host side
# --------------------------------------------------------------------------
def _wrap16(ix):
    """dma_gather int16 index layout: [16, n/16] wrapped, tiled to 128 parts."""
    return np.tile(ix.reshape(-1, 16).T, (8, 1))


def _balance(dst_all, src_all):
    """Assign each core's nodes to NBLK blocks (<=128 each) so that every
    (block, src-chunk-region) in-degree bucket stays <= 4*128.  A node's
    chunk region (for its outgoing edges) is its original quarter of the
    core slice, so regions are fixed before balancing and the per-core
    problems decouple."""
    region_of = (np.arange(N_NODES) % SLICE) // REG_N
    q_src = region_of[src_all]
    core_of = dst_all // SLICE
    pos_of = np.zeros((N_CORES, SLICE), np.int64)
    for c in range(N_CORES):
        sel = core_of == c
        d_loc = dst_all[sel] - c * SLICE
        deg = np.zeros((SLICE, NCHUNK), np.int32)
        np.add.at(deg, (d_loc, q_src[sel]), 1)
        for ch in range(NCHUNK):
            nodes = np.arange(ch * REG_N, (ch + 1) * REG_N)
            dv = deg[nodes]
            order = np.argsort(-dv.sum(axis=1), kind="stable")
            loads = np.zeros((RB, NCHUNK), np.int64)
            cnts = np.zeros(RB, np.int64)
            for n in order:
                score = (loads + dv[n]).max(axis=1) * 1024 + cnts
                score[cnts >= P] = 1 << 60
                bb = int(np.argmin(score))
                pos_of[c][nodes[n]] = (ch * RB + bb) * P + cnts[bb]
                loads[bb] += dv[n]
                cnts[bb] += 1
    return pos_of


def _preprocess(inputs):
    x = np.asarray(inputs["x"], dtype=np.float32)
    ei = np.asarray(inputs["edge_index"]).astype(np.int64)
    batch = np.asarray(inputs["batch"]).astype(np.int64)
    gf = np.asarray(inputs["global_feat"], dtype=np.float32)
    W = {k: np.ascontiguousarray(np.asarray(inputs[k], dtype=np.float32))
         for k in ("Wg", "bg", "Wc", "bc", "Wm", "bm", "Ws", "bs", "Wo", "bo")}

    src_all, dst_all = ei[0], ei[1]
    pos_of = _balance(dst_all, src_all)
    # src -> (chunk bucket, row within that chunk's m_full tensor)
    s_core = src_all // SLICE
    s_pos = pos_of[s_core, src_all % SLICE]
    s_blk = s_pos // P
    s_ch = s_blk // RB
    s_row = (s_core * (RB * P) + (s_blk - s_ch * RB) * P
             + s_pos % P).astype(np.int16)
    core_of = dst_all // SLICE

    per_core = []
    counts = np.zeros((N_CORES, NBLK, NCHUNK), np.int64)
    for c in range(N_CORES):
        sel = np.nonzero(core_of == c)[0]
        d_pos = pos_of[c][dst_all[sel] - c * SLICE]
        blk = d_pos // P
        q = s_ch[sel]
        key = blk * NCHUNK + q
        order = np.argsort(key, kind="stable")
        sel, key = sel[order], key[order]
        cnt = np.bincount(key, minlength=NBLK * NCHUNK).reshape(NBLK, NCHUNK)
        counts[c] = cnt
        per_core.append((key, (d_pos[order] % P).astype(np.float32),
                         s_row[sel], cnt))

    k_used = (-(-counts.max(axis=0) // P)).astype(np.int64)  # [NBLK, NCHUNK]
    k_used += k_used & 1          # DoubleRow pairs col-tiles: keep k even
    tch = k_used.sum(axis=1)
    boff = np.concatenate([[0], np.cumsum(tch)])
    TOTCH = int(boff[-1])
    TCHMAX = int(tch.max())

    # static call layout: per (group, bucket) capacity and flat offsets
    gcap = np.zeros((NGRP, NCHUNK), np.int64)
    for g in range(NGRP):
        gcap[g] = k_used[g * G_BLKS:(g + 1) * G_BLKS].sum(axis=0)
    # flat idx-slot offset of bucket (b, q) inside the concatenated calls
    bucket_off = np.zeros((NBLK, NCHUNK), np.int64)
    off = 0
    for g in range(NGRP):
        for q in range(NCHUNK):
            for j in range(G_BLKS):
                b = g * G_BLKS + j
                bucket_off[b][q] = off
                off += k_used[b][q] * P
    TOTSLOT = int(off)

    iota_np = np.tile(np.arange(P, dtype=np.float32).astype(np_bf16),
                      (P, TCHMAX))
    shared = {
        "gfT": np.ascontiguousarray(gf.T),
        "iota": iota_np,
        "idb": np.eye(P, dtype=np.float32).astype(np_bf16),
        "Wg": W["Wg"],
        "Wc1": np.ascontiguousarray(W["Wc"][:IN_LOCAL]).astype(np_bf16),
        "Wc2": np.ascontiguousarray(W["Wc"][IN_LOCAL:]),
        "Wm": W["Wm"].astype(np_bf16),
        "Ws": W["Ws"].astype(np_bf16),
        "Wo": W["Wo"].astype(np_bf16),
        "bg_c": W["bg"].reshape(HIDDEN, 1),
        "bc_b": np.tile(W["bc"], (P, 1)),
        "bm_r": W["bm"].reshape(1, HIDDEN).astype(np_bf16),
        "ones_r": np.ones((1, P), np.float32).astype(np_bf16),
        "bs_c": W["bs"].reshape(HIDDEN, 1),
        "bo_c": W["bo"].reshape(NUM_CLASSES, 1),
    }

    in_maps = []
    for c in range(N_CORES):
        key, d128, l16, cnt = per_core[c]
        ix_pad = np.zeros(TOTSLOT, np.int16)
        dst_pad = np.full(TOTCH * P, -1.0, np.float32)
        flat_cnt = cnt.reshape(-1)
        cum = np.cumsum(flat_cnt) - flat_cnt
        within = np.arange(len(key)) - np.repeat(cum, flat_cnt)
        # gather-call slot (group-major) and dstT slot (block-major)
        pos_ix = np.repeat(bucket_off.reshape(-1), flat_cnt) + within
        dst_off = (boff[:-1][:, None]
                   + np.concatenate([np.zeros((NBLK, 1), np.int64),
                                     np.cumsum(k_used, axis=1)[:, :-1]],
                                    axis=1)) * P
        pos_dst = np.repeat(dst_off.reshape(-1), flat_cnt) + within
        ix_pad[pos_ix] = l16
        dst_pad[pos_dst] = d128
        ixe = np.concatenate(
            [_wrap16(ix_pad[bucket_off[g * G_BLKS][q]:
                            bucket_off[g * G_BLKS][q] + gcap[g][q] * P])
             for g in range(NGRP) for q in range(NCHUNK)], axis=1)
        dstT = np.ascontiguousarray(
            dst_pad.reshape(-1, P).T).astype(np_bf16)

        # u[batch] one-hots: per region a 128-graph window (base per core,
        # encoded in SEL), per node-slot a one-hot of batch - base.
        bpad = np.zeros(PAD_SLICE, np.int64)
        bpad[pos_of[c]] = batch[c * SLICE:(c + 1) * SLICE]
        real = np.zeros(PAD_SLICE, bool)
        real[pos_of[c]] = True
        ou = np.zeros((P, PAD_SLICE), np.int8)
        sel_oh = np.zeros((P, NCHUNK, GBLK, P), np.int8)
        for r in range(NCHUNK):
            n0 = c * SLICE + r * REG_N
            base = min(int(batch[n0]), N_GRAPHS - P)
            ssl = slice(r * RB * P, (r + 1) * RB * P)
            rel = bpad[ssl] - base
            rr = real[ssl]
            assert (rel[rr] >= 0).all() and (rel[rr] < P).all(), \
                f"core {c} region {r}: batch window exceeds {P} graphs"
            cols = np.arange(ssl.start, ssl.stop)[rr]
            ou[rel[rr], cols] = 1
            # SEL[grow, r, gt, w] = 1 iff global graph gt*128+grow == base+w
            w = np.arange(P)
            gabs = base + w
            sel_oh[gabs % P, r, gabs // P, w] = 1

        xT = np.zeros((IN_LOCAL, PAD_SLICE), np_bf16)
        xT[:, pos_of[c]] = x[c * SLICE:(c + 1) * SLICE].T.astype(np_bf16)

        m = dict(shared)
        m.update({"xT": xT, "OU": ou.astype(np_bf16),
                  "SEL": sel_oh.reshape(P, NCHUNK * GBLK * P).astype(np_bf16),
                  "ixe": ixe, "dstT": dstT})
        in_maps.append(m)
    return k_used, pos_of, in_maps


# --------------------------------------------------------------------------
# profiling hook (NTFF via the axon PJRT .so; absent module in this image)
# --------------------------------------------------------------------------
def _profile_hook():
    so = "/opt/axon/libaxon_pjrt.so"
    if not os.path.exists(so):
        return None
    lib = ctypes.CDLL(so)
    if not hasattr(lib, "axon_start_nrt_profile"):
        return None
    lib.axon_start_nrt_profile.argtypes = [ctypes.POINTER(ctypes.c_int64),
                                           ctypes.c_size_t]
    lib.axon_start_nrt_profile.restype = ctypes.c_int64
    lib.axon_stop_nrt_profile.argtypes = [ctypes.c_char_p]
    lib.axon_stop_nrt_profile.restype = ctypes.c_int64

    @contextlib.contextmanager
    def hook(output_dir, device_ids):
        import jax
        jax.devices()
        if device_ids:
            ids = (ctypes.c_int64 * len(device_ids))(*device_ids)
            rc = lib.axon_start_nrt_profile(ids, len(device_ids))
        else:
            rc = lib.axon_start_nrt_profile(None, 0)
        if rc != 0:
            raise RuntimeError(f"axon_start_nrt_profile rc={rc}")
        try:
            yield
        finally:
            n = lib.axon_stop_nrt_profile(str(output_dir).encode())
            print(f"profile: {n} file(s) written to {output_dir}",
                  file=sys.stderr)

    return hook


def _run(nc, in_maps):
    from concourse import bass2jax
    trace_dir = os.environ.get("GNN_TRACE_DIR", "")
    if not trace_dir:
        return bass2jax.run_bass_via_pjrt(nc, in_maps, n_cores=N_CORES)
    hook = _profile_hook()
    if hook is None:
        return bass2jax.run_bass_via_pjrt(nc, in_maps, n_cores=N_CORES)
    import time as _time
    trace_dir = os.path.join(trace_dir, f"run_{int(_time.time()*1000)}")
    os.makedirs(trace_dir, exist_ok=True)
    last_run["trace_dir"] = trace_dir
    trace_cores = [int(t) for t in
                   os.environ.get("GNN_TRACE_CORES", "0").split(",")]
    with hook(trace_dir, trace_cores):
        results = bass2jax.run_bass_via_pjrt(nc, in_maps, n_cores=N_CORES)
    try:
        from concourse._compat import FishPath
        import gauge.profiler as gprof
        profile = gprof.Profile(
            profile_path=FishPath(trace_dir), kernel_dev_mode=True,
            profile_on_exit=False, bass_kernel=nc.m,
            offline_processing=True, fname="*_body*")
        profile.convert_ntffs_to_json(tuple(trace_cores))
        j = profile.load_json(trace_cores[0])
        last_run["summary"] = j["summary"][0] if j else None
        last_run["exec_time_ns"] = (
            int(j["summary"][0]["total_time"] * 1e9) if j else None)
        last_run["profile_json"] = str(profile.json_path(trace_cores[0]))
    except Exception as e:  # profiling must never break the run
        print(f"profile post-processing failed: {e}", file=sys.stderr)
    return results


def kernel(**inputs) -> np.ndarray:
    k_used, pos_of, in_maps = _preprocess(inputs)
    key = k_used.tobytes()
    nc = _prog_cache.get(key)
    if nc is None:
        nc = _build(k_used)
        _prog_cache[key] = nc
    last_run.clear()
    results = _run(nc, in_maps)
    out = np.empty((N_NODES, NUM_CLASSES), np.float32)
    for c in range(N_CORES):
        out[c * SLICE:(c + 1) * SLICE] = results[c]["outT"].T[pos_of[c]]
    return out
